# revision 16
# baseline (speedup 1.0000x reference)
"""MoE layer (E=8, top-2, SwiGLU experts) on 8 trn2 NeuronCores.

Strategy (expert-pair x tensor-parallel hybrid, host-routed):
  - Router on host in fp32 (exactly matches the reference's selection).
  - The 8 experts are paired big-with-small by routed-token count; each of
    the 4 pairs gets 2 cores. Core (pair p, half h) processes BOTH experts
    of its pair against its OWN 1024-row hidden half:
        g = w1h.T @ x; u = w3h.T @ x; h = silu(g)*u; y_partial = w2h.T @ h
    and the host sums the two halves' partials. Every core runs the same
    program over [C1 | C2] token groups (C1 = max big-expert count, C2 =
    max small-expert count over pairs), so per-core work = (C1+C2)/2 full-H
    token-pairs ~= 1052 -- below the 1080 an expert-per-core split pays for
    the most-loaded expert -- while x is only duplicated 2x (per-core DMA
    ~21 MB, same regime as expert-parallel).
  - All tensors pre-transposed and pre-packed on host so every device DMA
    is a pure linear copy.

Compute bf16 (fp32 PSUM accumulation); output partials bf16 (adds ~0.1%
rel err, negligible vs the 2e-2 gate).
"""

import os
import numpy as np
import ml_dtypes

B, S, D, H, E = 2, 2048, 1024, 2048, 8
T = B * S
TOP_K = 2
P = 128
NTOK = 512    # max matmul free dim / one PSUM bank of fp32
D_T = D // P  # 8 contraction slabs for stage 1 / output slabs for stage 2
TP = 2        # hidden-dim shards per expert pair
HS = H // TP  # per-core hidden slice
HS_T = HS // P  # 8 hidden slabs per expert slot
NPAIR = E // 2

_cache = {}

# set by the last kernel() call when tracing is enabled (KERNEL_TRACE=1)
LAST_RESULTS = None


def _chunk_sizes(ge):
    """First chunk as large as possible (a longer chunk 0 spreads the
    expert's 6.3MB weight-set consumption over more compute, keeping demand
    under the DMA ring bandwidth); remainder split evenly in multiples of 8.
    Every chunk stays >=256 for ge>=1024 so the free dim hides LDWEIGHTS."""
    first = min(NTOK, ge)
    sizes = [first]
    rem = ge - first
    if rem:
        k = -(-rem // NTOK)
        base, r8 = divmod(rem // 8, k)
        sizes += [(base + (1 if i < r8 else 0)) * 8 for i in range(k)]
    return sizes


def _pack_x(xT, chunks):
    """[D, cap] -> [128, D_T*cap], chunk-blocked, partition-major."""
    arr = xT.reshape(D_T, P, -1).transpose(1, 0, 2)   # [128, D_T, cap]
    blocks = []
    s0 = 0
    for n in chunks:
        blocks.append(arr[:, :, s0:s0 + n].reshape(P, D_T * n))
        s0 += n
    return np.ascontiguousarray(np.concatenate(blocks, axis=1))


def _build_nc(caps, act="silu"):
    """caps = (C1, C2): the two per-core group capacities."""
    import concourse.mybir as mybir
    import concourse.tile as tile
    from concourse import bacc

    bf16 = mybir.dt.bfloat16
    f32 = mybir.dt.float32
    Silu = (
        mybir.ActivationFunctionType.Silu
        if act == "silu"
        else mybir.ActivationFunctionType.Sigmoid
    )

    group_chunks = [_chunk_sizes(c) for c in caps]
    capall = sum(caps)

    nc = bacc.Bacc()
    xT_d = nc.declare_dram_parameter("xT", [P, D_T * capall], bf16, isOutput=False)
    # per (slot, h-slab, proj) weight pack, stage-1 consumption order
    w13_d = nc.declare_dram_parameter(
        "w13T", [2 * HS_T * 2, P, D_T * P], bf16, isOutput=False)
    w2_d = nc.declare_dram_parameter("w2T", [2, P, HS_T * D], bf16, isOutput=False)
    yT_d = nc.declare_dram_parameter("yT", [D, capall], bf16, isOutput=True)

    with tile.TileContext(nc) as tc:
        with (
            tc.tile_pool(name="wpool", bufs=1) as wpool,
            tc.tile_pool(name="xpool", bufs=2) as xpool,
            tc.tile_pool(name="hpool", bufs=2) as hpool,
            tc.tile_pool(name="gpool", bufs=4) as gpool,
            tc.tile_pool(name="opool", bufs=4) as opool,
            tc.tile_pool(name="pspool", bufs=2, space="PSUM") as pspool,
        ):
            chunk_list = []   # (slot, col offset, n)
            off = 0
            for slot, ch in enumerate(group_chunks):
                for n in ch:
                    chunk_list.append((slot, off, n))
                    off += n
            n_chunks = len(chunk_list)
            # Process slot 0's last (smallest) chunk LAST: the final PSUM
            # copy + output DMA serialize after the last matmul, so the
            # closing chunk should have the smallest free dim. Offsets into
            # xT_d/yT_d are explicit, so reordering is free.
            if len(group_chunks[0]) >= 2:
                chunk_list = (chunk_list[:len(group_chunks[0]) - 1]
                              + chunk_list[len(group_chunks[0]):]
                              + [chunk_list[len(group_chunks[0]) - 1]])

            # ALL inputs are loaded up front in consumption order and stay
            # SBUF-resident (no pool recycling): any mid-stream DMA hiccup
            # would stall the PE and -- via a HAM re-throttle -- cost ~2x the
            # stall, so the steady-state stream must depend on no DMA at all
            # (outputs ride the separate ACT ring).
            def load_x(idx):
                slot, s0, n = chunk_list[idx]
                xt = xpool.tile([P, D_T, n], bf16, tag=f"x_{idx}", name=f"x_{idx}")
                nc.sync.dma_start(xt[:], xT_d[:, D_T * s0:D_T * (s0 + n)])
                return xt

            def load_w13_pack(slot, s, pidx, ring=None):
                t = wpool.tile([P, D_T, P], bf16,
                               tag=f"w13_{slot}_{s}_{pidx}",
                               name=f"w13_{slot}_{s}_{pidx}")
                (ring or nc.sync).dma_start(
                    t[:].rearrange("p d c -> p (d c)"),
                    w13_d[slot * HS_T * 2 + s * 2 + pidx])
                return t

            def load_w2(slot):
                t2 = wpool.tile([P, HS_T, D], bf16,
                                tag=f"w2_{slot}", name=f"w2_{slot}")
                nc.sync.dma_start(t2[:].rearrange("p d c -> p (d c)"), w2_d[slot])
                return t2

            # The first gate group's critical prefix (all of chunk-0's x +
            # the first w1 pack) is balanced across BOTH DGE rings -- 768KB
            # on SP (x slabs 0-5) and 768KB on ACT (w1 pack, x slabs 6-7,
            # w3 pack; the ACT ring is otherwise idle until outputs begin).
            # Slabs 6-7 are consumed last within the gate group, so their
            # later arrival behind the w1 pack is off the critical path.
            n0 = chunk_list[0][2]
            xs0 = xpool.tile([P, D_T, n0], bf16, tag="x_0", name="x_0")
            nc.sync.dma_start(xs0[:, :3, :], xT_d[:, 0:3 * n0])
            w13t = {0: [load_w13_pack(0, 0, 0, ring=nc.scalar)]}
            nc.sync.dma_start(xs0[:, 3:6, :], xT_d[:, 3 * n0:6 * n0])
            nc.scalar.dma_start(xs0[:, 6:, :], xT_d[:, 6 * n0:D_T * n0])
            w13t[0].append(load_w13_pack(0, 0, 1, ring=nc.scalar))
            for s in range(1, HS_T):
                for pidx in range(2):
                    w13t[0].append(load_w13_pack(0, s, pidx))
            w2t = {0: load_w2(0)}
            xtiles = {0: xs0}
            # remaining x chunks in consumption order, slot 1's weights
            # queued just before its first chunk's x
            for idx in range(1, n_chunks):
                if chunk_list[idx][0] == 1 and 1 not in w13t:
                    w13t[1] = [load_w13_pack(1, s, pidx)
                               for s in range(HS_T) for pidx in range(2)]
                    w2t[1] = load_w2(1)
                xtiles[idx] = load_x(idx)

            # HAM warmup: full-width (N=512) matmuls on zeros from right
            # after the framework preamble until the first chunk's data
            # lands. The PE clock gate (HAM) only un-throttles after ~3-5us
            # of SUSTAINED high-duty matmul activity -- tiny matmuls don't
            # register as busy -- so the warmup stream must look real.
            warm_sb = gpool.tile([P, NTOK], bf16, tag="warm_sb", name="warm_sb")
            nc.gpsimd.memset(warm_sb[:], 0.0)
            warm_ps = pspool.tile([P, NTOK], f32, tag="py", name="warm_ps", bufs=3)
            for _ in range(14):
                nc.tensor.matmul(
                    warm_ps[:16, :], lhsT=warm_sb[:, :16], rhs=warm_sb[:],
                    start=True, stop=True,
                )

            for idx, (slot, s0, n) in enumerate(chunk_list):
                xs = xtiles[idx]
                w13 = w13t[slot]
                t2 = w2t[slot]

                # stage 1: hT[s] = silu(w1s.T@xT) * (w3s.T@xT)  [128, n] bf16
                hts = []
                for s in range(HS_T):
                    t1 = w13[s * 2]
                    t3 = w13[s * 2 + 1]
                    pg = pspool.tile([P, NTOK], f32, tag="pg", name="pg")
                    for d in range(D_T):
                        nc.tensor.matmul(
                            pg[:, :n], lhsT=t1[:, d, :], rhs=xs[:, d, :],
                            start=(d == 0), stop=(d == D_T - 1),
                        )
                    pu = pspool.tile([P, NTOK], f32, tag="pu", name="pu", bufs=3)
                    for d in range(D_T):
                        nc.tensor.matmul(
                            pu[:, :n], lhsT=t3[:, d, :], rhs=xs[:, d, :],
                            start=(d == 0), stop=(d == D_T - 1),
                        )
                    g = gpool.tile([P, NTOK], bf16, tag="g", name="g")
                    nc.scalar.activation(g[:, :n], pg[:, :n], Silu)
                    ht = hpool.tile([P, NTOK], bf16, tag=f"h_{s}", name=f"h_{s}")
                    nc.vector.tensor_mul(out=ht[:, :n], in0=g[:, :n], in1=pu[:, :n])
                    hts.append(ht)

                # stage 2: yT[do] = sum_s w2s[s,do].T @ hT[s]  [128, n] bf16
                # All outputs ride the ACT ring -- it stays warm from
                # streaming outputs all along, while the SP ring is cold-idle
                # by the last chunk and would pay ~2us of DGE re-spin-up.
                for do in range(D_T):
                    py = pspool.tile([P, NTOK], f32, tag="py", name="py", bufs=3)
                    for s in range(HS_T):
                        nc.tensor.matmul(
                            py[:, :n],
                            lhsT=t2[:, s, do * P:(do + 1) * P],
                            rhs=hts[s][:, :n],
                            start=(s == 0), stop=(s == HS_T - 1),
                        )
                    # bufs=8: the up-front input flood keeps the 16 HW DMA
                    # queues busy for the first ~55us, starving the ACT
                    # ring's output DMAs; a full chunk of copies must be able
                    # to land without waiting on any output-DMA completion,
                    # else the py->copy->ot chain backpressures the PE.
                    ot = opool.tile([P, NTOK], bf16, tag="o", name="o", bufs=8)
                    nc.vector.tensor_copy(ot[:, :n], py[:, :n])
                    nc.scalar.dma_start(yT_d[do * P:(do + 1) * P, s0:s0 + n], ot[:, :n])

    nc.finalize()
    return nc


def kernel(x, router_w, w1, w2, w3):
    global LAST_RESULTS
    from concourse.bass_utils import run_bass_kernel_spmd

    x = np.ascontiguousarray(np.asarray(x, dtype=np.float32))
    router_w = np.asarray(router_w, dtype=np.float32)
    flat = x.reshape(T, D)

    # ---- host router (fp32, matches reference math) ----
    logits = flat @ router_w.T                      # [T, E]
    rows = np.arange(T)
    i1 = np.argmax(logits, axis=1)
    v1 = logits[rows, i1]
    masked = logits.copy()
    masked[rows, i1] = -np.inf
    i2 = np.argmax(masked, axis=1)
    v2 = masked[rows, i2]
    e2 = np.exp(v2 - v1)
    wt1 = 1.0 / (1.0 + e2)
    wt2 = e2 / (1.0 + e2)

    # ---- dispatch: token lists per expert ----
    idxs, wts = [], []
    for e in range(E):
        m1 = i1 == e
        m2 = i2 == e
        idx = np.nonzero(m1 | m2)[0]
        w = np.where(m1[idx], wt1[idx], wt2[idx]).astype(np.float32)
        idxs.append(idx)
        wts.append(w)
    cnts = np.array([len(i) for i in idxs])

    # pair big-with-small so both group capacities stay tight
    order = np.argsort(-cnts)
    pairs = [(int(order[i]), int(order[E - 1 - i])) for i in range(NPAIR)]
    C1 = -(-int(cnts[order[0]]) // 8) * 8
    C2 = -(-int(max(cnts[e2_] for _, e2_ in pairs)) // 8) * 8
    caps = (C1, C2)

    if caps not in _cache:
        _cache[caps] = _build_nc(caps)
    nc = _cache[caps]

    # ---- inputs (bf16, pre-transposed, pre-packed) ----
    bf = ml_dtypes.bfloat16
    chunks = _chunk_sizes(C1) + _chunk_sizes(C2)
    xpacks = []
    for eA, eB in pairs:
        xT = np.zeros((D, C1 + C2), dtype=bf)
        xT[:, :cnts[eA]] = flat[idxs[eA]].T.astype(bf)
        xT[:, C1:C1 + cnts[eB]] = flat[idxs[eB]].T.astype(bf)
        xpacks.append(_pack_x(xT, chunks))

    in_maps = []
    for c in range(E):
        p, hh = c // TP, c % TP
        r0 = hh * HS
        w13 = np.empty((2 * HS_T * 2, P, D_T * P), dtype=bf)
        w2s = np.empty((2, P, HS_T * D), dtype=bf)
        for slot, e in enumerate(pairs[p]):
            w1eT = np.ascontiguousarray(w1[e][r0:r0 + HS].T).astype(bf)  # [D, HS]
            w3eT = np.ascontiguousarray(w3[e][r0:r0 + HS].T).astype(bf)
            a1 = w1eT.reshape(D_T, P, HS).transpose(1, 0, 2)   # [128, D_T, HS]
            a3 = w3eT.reshape(D_T, P, HS).transpose(1, 0, 2)
            for s in range(HS_T):
                w13[slot * HS_T * 2 + s * 2 + 0] = \
                    a1[:, :, s * P:(s + 1) * P].reshape(P, D_T * P)
                w13[slot * HS_T * 2 + s * 2 + 1] = \
                    a3[:, :, s * P:(s + 1) * P].reshape(P, D_T * P)
            w2eT = np.ascontiguousarray(w2[e].T[r0:r0 + HS]).astype(bf)  # [HS, D]
            w2s[slot] = w2eT.reshape(HS_T, P, D).transpose(1, 0, 2).reshape(P, HS_T * D)
        in_maps.append({"xT": xpacks[p], "w13T": w13, "w2T": w2s})

    trace = os.environ.get("KERNEL_TRACE", "0") == "1"
    kwargs = {}
    if trace:
        kwargs = dict(trace=True, trace_cores=list(range(E)))
    res = run_bass_kernel_spmd(nc, in_maps, core_ids=list(range(E)), **kwargs)
    LAST_RESULTS = res

    # ---- combine: sum the two hidden-half partials, weight, scatter ----
    out = np.zeros((T, D), dtype=np.float32)
    for p, (eA, eB) in enumerate(pairs):
        ysum = (res.results[p * TP]["yT"].astype(np.float32)
                + res.results[p * TP + 1]["yT"].astype(np.float32))
        out[idxs[eA]] += wts[eA][:, None] * ysum[:, :cnts[eA]].T
        out[idxs[eB]] += wts[eB][:, None] * ysum[:, C1:C1 + cnts[eB]].T
    return out.reshape(B, S, D)


# revision 18
# speedup vs baseline: 1.0026x; 1.0026x over previous
"""MoE layer (E=8, top-2, SwiGLU experts) on 8 trn2 NeuronCores.

Strategy (expert-pair x tensor-parallel hybrid, host-routed):
  - Router on host in fp32 (exactly matches the reference's selection).
  - The 8 experts are paired big-with-small by routed-token count; each of
    the 4 pairs gets 2 cores. Core (pair p, half h) processes BOTH experts
    of its pair against its OWN 1024-row hidden half:
        g = w1h.T @ x; u = w3h.T @ x; h = silu(g)*u; y_partial = w2h.T @ h
    and the host sums the two halves' partials. Every core runs the same
    program over [C1 | C2] token groups (C1 = max big-expert count, C2 =
    max small-expert count over pairs), so per-core work = (C1+C2)/2 full-H
    token-pairs ~= 1052 -- below the 1080 an expert-per-core split pays for
    the most-loaded expert -- while x is only duplicated 2x (per-core DMA
    ~21 MB, same regime as expert-parallel).
  - All tensors pre-transposed and pre-packed on host so every device DMA
    is a pure linear copy.

Compute bf16 (fp32 PSUM accumulation); output partials bf16 (adds ~0.1%
rel err, negligible vs the 2e-2 gate).
"""

import os
import numpy as np
import ml_dtypes

B, S, D, H, E = 2, 2048, 1024, 2048, 8
T = B * S
TOP_K = 2
P = 128
NTOK = 512    # max matmul free dim / one PSUM bank of fp32
D_T = D // P  # 8 contraction slabs for stage 1 / output slabs for stage 2
TP = 2        # hidden-dim shards per expert pair
HS = H // TP  # per-core hidden slice
HS_T = HS // P  # 8 hidden slabs per expert slot
NPAIR = E // 2

_cache = {}

# set by the last kernel() call when tracing is enabled (KERNEL_TRACE=1)
LAST_RESULTS = None


def _chunk_sizes(ge):
    """First chunk as large as possible (a longer chunk 0 spreads the
    expert's 6.3MB weight-set consumption over more compute, keeping demand
    under the DMA ring bandwidth); remainder split evenly in multiples of 8.
    Every chunk stays >=256 for ge>=1024 so the free dim hides LDWEIGHTS."""
    first = min(NTOK, ge)
    sizes = [first]
    rem = ge - first
    if rem:
        k = -(-rem // NTOK)
        base, r8 = divmod(rem // 8, k)
        sizes += [(base + (1 if i < r8 else 0)) * 8 for i in range(k)]
    return sizes


def _pack_x(xT, chunks):
    """[D, cap] -> [128, D_T*cap], chunk-blocked, partition-major."""
    arr = xT.reshape(D_T, P, -1).transpose(1, 0, 2)   # [128, D_T, cap]
    blocks = []
    s0 = 0
    for n in chunks:
        blocks.append(arr[:, :, s0:s0 + n].reshape(P, D_T * n))
        s0 += n
    return np.ascontiguousarray(np.concatenate(blocks, axis=1))


def _build_nc(caps, act="silu"):
    """caps = (C1, C2): the two per-core group capacities."""
    import concourse.mybir as mybir
    import concourse.tile as tile
    from concourse import bacc

    bf16 = mybir.dt.bfloat16
    f32 = mybir.dt.float32
    Silu = (
        mybir.ActivationFunctionType.Silu
        if act == "silu"
        else mybir.ActivationFunctionType.Sigmoid
    )

    group_chunks = [_chunk_sizes(c) for c in caps]
    capall = sum(caps)

    nc = bacc.Bacc()
    xT_d = nc.declare_dram_parameter("xT", [P, D_T * capall], bf16, isOutput=False)
    # per (slot, h-slab, proj) weight pack, stage-1 consumption order
    w13_d = nc.declare_dram_parameter(
        "w13T", [2 * HS_T * 2, P, D_T * P], bf16, isOutput=False)
    w2_d = nc.declare_dram_parameter("w2T", [2, P, HS_T * D], bf16, isOutput=False)
    yT_d = nc.declare_dram_parameter("yT", [D, capall], bf16, isOutput=True)

    with tile.TileContext(nc) as tc:
        with (
            tc.tile_pool(name="wpool", bufs=1) as wpool,
            tc.tile_pool(name="xpool", bufs=2) as xpool,
            tc.tile_pool(name="hpool", bufs=2) as hpool,
            tc.tile_pool(name="gpool", bufs=4) as gpool,
            tc.tile_pool(name="opool", bufs=4) as opool,
            tc.tile_pool(name="pspool", bufs=2, space="PSUM") as pspool,
        ):
            chunk_list = []   # (slot, col offset, n)
            off = 0
            for slot, ch in enumerate(group_chunks):
                for n in ch:
                    chunk_list.append((slot, off, n))
                    off += n
            n_chunks = len(chunk_list)
            # Process slot 0's last (smallest) chunk LAST: the final PSUM
            # copy + output DMA serialize after the last matmul, so the
            # closing chunk should have the smallest free dim. Offsets into
            # xT_d/yT_d are explicit, so reordering is free.
            if len(group_chunks[0]) >= 2:
                chunk_list = (chunk_list[:len(group_chunks[0]) - 1]
                              + chunk_list[len(group_chunks[0]):]
                              + [chunk_list[len(group_chunks[0]) - 1]])

            # ALL inputs are loaded up front in consumption order and stay
            # SBUF-resident (no pool recycling): any mid-stream DMA hiccup
            # would stall the PE and -- via a HAM re-throttle -- cost ~2x the
            # stall, so the steady-state stream must depend on no DMA at all
            # (outputs ride the separate ACT ring).
            def load_x(idx):
                slot, s0, n = chunk_list[idx]
                xt = xpool.tile([P, D_T, n], bf16, tag=f"x_{idx}", name=f"x_{idx}")
                nc.sync.dma_start(xt[:], xT_d[:, D_T * s0:D_T * (s0 + n)])
                return xt

            def load_w13_pack(slot, s, pidx, ring=None):
                t = wpool.tile([P, D_T, P], bf16,
                               tag=f"w13_{slot}_{s}_{pidx}",
                               name=f"w13_{slot}_{s}_{pidx}")
                (ring or nc.sync).dma_start(
                    t[:].rearrange("p d c -> p (d c)"),
                    w13_d[slot * HS_T * 2 + s * 2 + pidx])
                return t

            def load_w2(slot):
                t2 = wpool.tile([P, HS_T, D], bf16,
                                tag=f"w2_{slot}", name=f"w2_{slot}")
                nc.sync.dma_start(t2[:].rearrange("p d c -> p (d c)"), w2_d[slot])
                return t2

            # The warmup zero-tile memset must be the FIRST gpsimd-queue op:
            # the prefix below also rides that queue (SWDGE), and the PE
            # warmup can only start after the memset.
            warm_sb = gpool.tile([P, NTOK], bf16, tag="warm_sb", name="warm_sb")
            nc.gpsimd.memset(warm_sb[:], 0.0)

            # The first gate group's critical prefix (all of chunk-0's x +
            # the first w1/w3 packs, 1.5MB) is balanced across THREE DGE
            # paths -- ~512KB each on the SP ring (x slabs 0-3), the ACT
            # ring (w1 pack, x slabs 6-7) and the gpsimd SWDGE path
            # (x slabs 4-5, w3 pack) -- the cold-start bottleneck is the
            # per-ring descriptor feed (~140 GB/s each), not the shared
            # SDMA engines. Slabs 6-7 are consumed last within the gate
            # group, so their arrival behind the w1 pack is off the
            # critical path.
            n0 = chunk_list[0][2]
            xs0 = xpool.tile([P, D_T, n0], bf16, tag="x_0", name="x_0")
            nc.sync.dma_start(xs0[:, :4, :], xT_d[:, 0:4 * n0])
            w13t = {0: [load_w13_pack(0, 0, 0, ring=nc.scalar)]}
            nc.gpsimd.dma_start(xs0[:, 4:6, :], xT_d[:, 4 * n0:6 * n0])
            nc.scalar.dma_start(xs0[:, 6:, :], xT_d[:, 6 * n0:D_T * n0])
            w13t[0].append(load_w13_pack(0, 0, 1, ring=nc.gpsimd))
            for s in range(1, HS_T):
                for pidx in range(2):
                    w13t[0].append(load_w13_pack(0, s, pidx))
            w2t = {0: load_w2(0)}
            xtiles = {0: xs0}
            # remaining x chunks in consumption order, slot 1's weights
            # queued just before its first chunk's x
            for idx in range(1, n_chunks):
                if chunk_list[idx][0] == 1 and 1 not in w13t:
                    w13t[1] = [load_w13_pack(1, s, pidx)
                               for s in range(HS_T) for pidx in range(2)]
                    w2t[1] = load_w2(1)
                xtiles[idx] = load_x(idx)

            # HAM warmup: full-width (N=512) matmuls on zeros from right
            # after the framework preamble until the first chunk's data
            # lands. The PE clock gate (HAM) only un-throttles after ~3-5us
            # of SUSTAINED high-duty matmul activity -- tiny matmuls don't
            # register as busy -- so the warmup stream must look real.
            # (warm_sb is memset at the top of the gpsimd queue, above.)
            warm_ps = pspool.tile([P, NTOK], f32, tag="py", name="warm_ps", bufs=3)
            for _ in range(14):
                nc.tensor.matmul(
                    warm_ps[:16, :], lhsT=warm_sb[:, :16], rhs=warm_sb[:],
                    start=True, stop=True,
                )

            for idx, (slot, s0, n) in enumerate(chunk_list):
                xs = xtiles[idx]
                w13 = w13t[slot]
                t2 = w2t[slot]

                # stage 1: hT[s] = silu(w1s.T@xT) * (w3s.T@xT)  [128, n] bf16
                hts = []
                for s in range(HS_T):
                    t1 = w13[s * 2]
                    t3 = w13[s * 2 + 1]
                    pg = pspool.tile([P, NTOK], f32, tag="pg", name="pg")
                    for d in range(D_T):
                        nc.tensor.matmul(
                            pg[:, :n], lhsT=t1[:, d, :], rhs=xs[:, d, :],
                            start=(d == 0), stop=(d == D_T - 1),
                        )
                    pu = pspool.tile([P, NTOK], f32, tag="pu", name="pu", bufs=3)
                    for d in range(D_T):
                        nc.tensor.matmul(
                            pu[:, :n], lhsT=t3[:, d, :], rhs=xs[:, d, :],
                            start=(d == 0), stop=(d == D_T - 1),
                        )
                    g = gpool.tile([P, NTOK], bf16, tag="g", name="g")
                    nc.scalar.activation(g[:, :n], pg[:, :n], Silu)
                    ht = hpool.tile([P, NTOK], bf16, tag=f"h_{s}", name=f"h_{s}")
                    nc.vector.tensor_mul(out=ht[:, :n], in0=g[:, :n], in1=pu[:, :n])
                    hts.append(ht)

                # stage 2: yT[do] = sum_s w2s[s,do].T @ hT[s]  [128, n] bf16
                # All outputs ride the ACT ring -- it stays warm from
                # streaming outputs all along, while the SP ring is cold-idle
                # by the last chunk and would pay ~2us of DGE re-spin-up.
                for do in range(D_T):
                    py = pspool.tile([P, NTOK], f32, tag="py", name="py", bufs=3)
                    for s in range(HS_T):
                        nc.tensor.matmul(
                            py[:, :n],
                            lhsT=t2[:, s, do * P:(do + 1) * P],
                            rhs=hts[s][:, :n],
                            start=(s == 0), stop=(s == HS_T - 1),
                        )
                    # bufs=8: the up-front input flood keeps the 16 HW DMA
                    # queues busy for the first ~55us, starving the ACT
                    # ring's output DMAs; a full chunk of copies must be able
                    # to land without waiting on any output-DMA completion,
                    # else the py->copy->ot chain backpressures the PE.
                    ot = opool.tile([P, NTOK], bf16, tag="o", name="o", bufs=8)
                    nc.vector.tensor_copy(ot[:, :n], py[:, :n])
                    nc.scalar.dma_start(yT_d[do * P:(do + 1) * P, s0:s0 + n], ot[:, :n])

    nc.finalize()
    return nc


def kernel(x, router_w, w1, w2, w3):
    global LAST_RESULTS
    from concourse.bass_utils import run_bass_kernel_spmd

    x = np.ascontiguousarray(np.asarray(x, dtype=np.float32))
    router_w = np.asarray(router_w, dtype=np.float32)
    flat = x.reshape(T, D)

    # ---- host router (fp32, matches reference math) ----
    logits = flat @ router_w.T                      # [T, E]
    rows = np.arange(T)
    i1 = np.argmax(logits, axis=1)
    v1 = logits[rows, i1]
    masked = logits.copy()
    masked[rows, i1] = -np.inf
    i2 = np.argmax(masked, axis=1)
    v2 = masked[rows, i2]
    e2 = np.exp(v2 - v1)
    wt1 = 1.0 / (1.0 + e2)
    wt2 = e2 / (1.0 + e2)

    # ---- dispatch: token lists per expert ----
    idxs, wts = [], []
    for e in range(E):
        m1 = i1 == e
        m2 = i2 == e
        idx = np.nonzero(m1 | m2)[0]
        w = np.where(m1[idx], wt1[idx], wt2[idx]).astype(np.float32)
        idxs.append(idx)
        wts.append(w)
    cnts = np.array([len(i) for i in idxs])

    # pair big-with-small so both group capacities stay tight
    order = np.argsort(-cnts)
    pairs = [(int(order[i]), int(order[E - 1 - i])) for i in range(NPAIR)]
    C1 = -(-int(cnts[order[0]]) // 8) * 8
    C2 = -(-int(max(cnts[e2_] for _, e2_ in pairs)) // 8) * 8
    caps = (C1, C2)

    if caps not in _cache:
        _cache[caps] = _build_nc(caps)
    nc = _cache[caps]

    # ---- inputs (bf16, pre-transposed, pre-packed) ----
    bf = ml_dtypes.bfloat16
    chunks = _chunk_sizes(C1) + _chunk_sizes(C2)
    xpacks = []
    for eA, eB in pairs:
        xT = np.zeros((D, C1 + C2), dtype=bf)
        xT[:, :cnts[eA]] = flat[idxs[eA]].T.astype(bf)
        xT[:, C1:C1 + cnts[eB]] = flat[idxs[eB]].T.astype(bf)
        xpacks.append(_pack_x(xT, chunks))

    in_maps = []
    for c in range(E):
        p, hh = c // TP, c % TP
        r0 = hh * HS
        w13 = np.empty((2 * HS_T * 2, P, D_T * P), dtype=bf)
        w2s = np.empty((2, P, HS_T * D), dtype=bf)
        for slot, e in enumerate(pairs[p]):
            w1eT = np.ascontiguousarray(w1[e][r0:r0 + HS].T).astype(bf)  # [D, HS]
            w3eT = np.ascontiguousarray(w3[e][r0:r0 + HS].T).astype(bf)
            a1 = w1eT.reshape(D_T, P, HS).transpose(1, 0, 2)   # [128, D_T, HS]
            a3 = w3eT.reshape(D_T, P, HS).transpose(1, 0, 2)
            for s in range(HS_T):
                w13[slot * HS_T * 2 + s * 2 + 0] = \
                    a1[:, :, s * P:(s + 1) * P].reshape(P, D_T * P)
                w13[slot * HS_T * 2 + s * 2 + 1] = \
                    a3[:, :, s * P:(s + 1) * P].reshape(P, D_T * P)
            w2eT = np.ascontiguousarray(w2[e].T[r0:r0 + HS]).astype(bf)  # [HS, D]
            w2s[slot] = w2eT.reshape(HS_T, P, D).transpose(1, 0, 2).reshape(P, HS_T * D)
        in_maps.append({"xT": xpacks[p], "w13T": w13, "w2T": w2s})

    trace = os.environ.get("KERNEL_TRACE", "0") == "1"
    kwargs = {}
    if trace:
        kwargs = dict(trace=True, trace_cores=list(range(E)))
    res = run_bass_kernel_spmd(nc, in_maps, core_ids=list(range(E)), **kwargs)
    LAST_RESULTS = res

    # ---- combine: sum the two hidden-half partials, weight, scatter ----
    out = np.zeros((T, D), dtype=np.float32)
    for p, (eA, eB) in enumerate(pairs):
        ysum = (res.results[p * TP]["yT"].astype(np.float32)
                + res.results[p * TP + 1]["yT"].astype(np.float32))
        out[idxs[eA]] += wts[eA][:, None] * ysum[:, :cnts[eA]].T
        out[idxs[eB]] += wts[eB][:, None] * ysum[:, C1:C1 + cnts[eB]].T
    return out.reshape(B, S, D)


# revision 19
# speedup vs baseline: 1.1926x; 1.1895x over previous
"""MoE layer (E=8, top-2, SwiGLU experts) on 8 trn2 NeuronCores.

Strategy (expert-pair x tensor-parallel hybrid, host-routed):
  - Router on host in fp32 (exactly matches the reference's selection).
  - The 8 experts are paired big-with-small by routed-token count; each of
    the 4 pairs gets 2 cores. Core (pair p, half h) processes BOTH experts
    of its pair against its OWN 1024-row hidden half:
        g = w1h.T @ x; u = w3h.T @ x; h = silu(g)*u; y_partial = w2h.T @ h
    and the host sums the two halves' partials. Every core runs the same
    program over [C1 | C2] token groups (C1 = max big-expert count, C2 =
    max small-expert count over pairs), so per-core work = (C1+C2)/2 full-H
    token-pairs ~= 1052 -- below the 1080 an expert-per-core split pays for
    the most-loaded expert -- while x is only duplicated 2x (per-core DMA
    ~21 MB, same regime as expert-parallel).
  - All tensors pre-transposed and pre-packed on host so every device DMA
    is a pure linear copy.

Compute bf16 (fp32 PSUM accumulation); output partials bf16 (adds ~0.1%
rel err, negligible vs the 2e-2 gate).
"""

import os
import numpy as np
import ml_dtypes

B, S, D, H, E = 2, 2048, 1024, 2048, 8
T = B * S
TOP_K = 2
P = 128
NTOK = 512    # max matmul free dim / one PSUM bank of fp32
D_T = D // P  # 8 contraction slabs for stage 1 / output slabs for stage 2
TP = 2        # hidden-dim shards per expert pair
HS = H // TP  # per-core hidden slice
HS_T = HS // P  # 8 hidden slabs per expert slot
NPAIR = E // 2

_cache = {}

# set by the last kernel() call when tracing is enabled (KERNEL_TRACE=1)
LAST_RESULTS = None


def _chunk_sizes(ge):
    """First chunk as large as possible (a longer chunk 0 spreads the
    expert's 6.3MB weight-set consumption over more compute, keeping demand
    under the DMA ring bandwidth); remainder split evenly in multiples of 8.
    Every chunk stays >=256 for ge>=1024 so the free dim hides LDWEIGHTS."""
    first = min(NTOK, ge)
    sizes = [first]
    rem = ge - first
    if rem:
        k = -(-rem // NTOK)
        base, r8 = divmod(rem // 8, k)
        sizes += [(base + (1 if i < r8 else 0)) * 8 for i in range(k)]
    return sizes


def _pack_x(xT, chunks):
    """[D, cap] -> [128, D_T*cap], chunk-blocked, partition-major."""
    arr = xT.reshape(D_T, P, -1).transpose(1, 0, 2)   # [128, D_T, cap]
    blocks = []
    s0 = 0
    for n in chunks:
        blocks.append(arr[:, :, s0:s0 + n].reshape(P, D_T * n))
        s0 += n
    return np.ascontiguousarray(np.concatenate(blocks, axis=1))


def _build_nc(caps, act="silu"):
    """caps = (C1, C2): the two per-core group capacities."""
    import concourse.mybir as mybir
    import concourse.tile as tile
    from concourse import bacc

    bf16 = mybir.dt.bfloat16
    f32 = mybir.dt.float32
    Silu = (
        mybir.ActivationFunctionType.Silu
        if act == "silu"
        else mybir.ActivationFunctionType.Sigmoid
    )

    group_chunks = [_chunk_sizes(c) for c in caps]
    capall = sum(caps)

    nc = bacc.Bacc()
    xT_d = nc.declare_dram_parameter("xT", [P, D_T * capall], bf16, isOutput=False)
    # per (slot, h-slab, proj) weight pack, stage-1 consumption order
    w13_d = nc.declare_dram_parameter(
        "w13T", [2 * HS_T * 2, P, D_T * P], bf16, isOutput=False)
    w2_d = nc.declare_dram_parameter("w2T", [2, P, HS_T * D], bf16, isOutput=False)
    yT_d = nc.declare_dram_parameter("yT", [D, capall], bf16, isOutput=True)

    with tile.TileContext(nc) as tc:
        with (
            tc.tile_pool(name="wpool", bufs=1) as wpool,
            tc.tile_pool(name="xpool", bufs=2) as xpool,
            tc.tile_pool(name="hpool", bufs=2) as hpool,
            tc.tile_pool(name="gpool", bufs=4) as gpool,
            tc.tile_pool(name="opool", bufs=4) as opool,
            tc.tile_pool(name="pspool", bufs=2, space="PSUM") as pspool,
        ):
            chunk_list = []   # (slot, col offset, n)
            off = 0
            for slot, ch in enumerate(group_chunks):
                for n in ch:
                    chunk_list.append((slot, off, n))
                    off += n
            n_chunks = len(chunk_list)
            # Process slot 0's last (smallest) chunk LAST: the final PSUM
            # copy + output DMA serialize after the last matmul, so the
            # closing chunk should have the smallest free dim. Offsets into
            # xT_d/yT_d are explicit, so reordering is free.
            if len(group_chunks[0]) >= 2:
                chunk_list = (chunk_list[:len(group_chunks[0]) - 1]
                              + chunk_list[len(group_chunks[0]):]
                              + [chunk_list[len(group_chunks[0]) - 1]])

            # ALL inputs are loaded up front in consumption order and stay
            # SBUF-resident (no pool recycling): any mid-stream DMA hiccup
            # would stall the PE and -- via a HAM re-throttle -- cost ~2x the
            # stall, so the steady-state stream must depend on no DMA at all
            # (outputs ride the separate ACT ring).
            def load_x(idx):
                slot, s0, n = chunk_list[idx]
                xt = xpool.tile([P, D_T, n], bf16, tag=f"x_{idx}", name=f"x_{idx}")
                nc.sync.dma_start(xt[:], xT_d[:, D_T * s0:D_T * (s0 + n)])
                return xt

            def load_w13_pack(slot, s, pidx, ring=None):
                t = wpool.tile([P, D_T, P], bf16,
                               tag=f"w13_{slot}_{s}_{pidx}",
                               name=f"w13_{slot}_{s}_{pidx}")
                (ring or nc.sync).dma_start(
                    t[:].rearrange("p d c -> p (d c)"),
                    w13_d[slot * HS_T * 2 + s * 2 + pidx])
                return t

            def load_w2(slot):
                t2 = wpool.tile([P, HS_T, D], bf16,
                                tag=f"w2_{slot}", name=f"w2_{slot}")
                nc.sync.dma_start(t2[:].rearrange("p d c -> p (d c)"), w2_d[slot])
                return t2

            # The first gate group's critical prefix (all of chunk-0's x +
            # the first w1 pack) is balanced across BOTH DGE rings -- 768KB
            # on SP (x slabs 0-5) and 768KB on ACT (w1 pack, x slabs 6-7,
            # w3 pack; the ACT ring is otherwise idle until outputs begin).
            # Slabs 6-7 are consumed last within the gate group, so their
            # later arrival behind the w1 pack is off the critical path.
            n0 = chunk_list[0][2]
            xs0 = xpool.tile([P, D_T, n0], bf16, tag="x_0", name="x_0")
            nc.sync.dma_start(xs0[:, :3, :], xT_d[:, 0:3 * n0])
            w13t = {0: [load_w13_pack(0, 0, 0, ring=nc.scalar)]}
            nc.sync.dma_start(xs0[:, 3:6, :], xT_d[:, 3 * n0:6 * n0])
            nc.scalar.dma_start(xs0[:, 6:, :], xT_d[:, 6 * n0:D_T * n0])
            w13t[0].append(load_w13_pack(0, 0, 1, ring=nc.scalar))
            for s in range(1, HS_T):
                for pidx in range(2):
                    w13t[0].append(load_w13_pack(0, s, pidx))
            w2t = {0: load_w2(0)}
            xtiles = {0: xs0}
            # remaining x chunks in consumption order, slot 1's weights
            # queued just before its first chunk's x
            for idx in range(1, n_chunks):
                if chunk_list[idx][0] == 1 and 1 not in w13t:
                    w13t[1] = [load_w13_pack(1, s, pidx)
                               for s in range(HS_T) for pidx in range(2)]
                    w2t[1] = load_w2(1)
                xtiles[idx] = load_x(idx)

            # HAM warmup: full-width (N=512) matmuls on zeros from right
            # after the framework preamble until the first chunk's data
            # lands. The PE clock gate (HAM) only un-throttles after ~3-5us
            # of SUSTAINED high-duty matmul activity -- tiny matmuls don't
            # register as busy -- so the warmup stream must look real.
            warm_sb = gpool.tile([P, NTOK], bf16, tag="warm_sb", name="warm_sb")
            nc.gpsimd.memset(warm_sb[:], 0.0)
            warm_ps = pspool.tile([P, NTOK], f32, tag="py", name="warm_ps", bufs=3)
            for _ in range(14):
                nc.tensor.matmul(
                    warm_ps[:16, :], lhsT=warm_sb[:, :16], rhs=warm_sb[:],
                    start=True, stop=True,
                )

            for idx, (slot, s0, n) in enumerate(chunk_list):
                xs = xtiles[idx]
                w13 = w13t[slot]
                t2 = w2t[slot]

                # stage 1: hT[s] = silu(w1s.T@xT) * (w3s.T@xT)  [128, n] bf16
                hts = []
                for s in range(HS_T):
                    t1 = w13[s * 2]
                    t3 = w13[s * 2 + 1]
                    pg = pspool.tile([P, NTOK], f32, tag="pg", name="pg")
                    for d in range(D_T):
                        nc.tensor.matmul(
                            pg[:, :n], lhsT=t1[:, d, :], rhs=xs[:, d, :],
                            start=(d == 0), stop=(d == D_T - 1),
                        )
                    pu = pspool.tile([P, NTOK], f32, tag="pu", name="pu", bufs=3)
                    for d in range(D_T):
                        nc.tensor.matmul(
                            pu[:, :n], lhsT=t3[:, d, :], rhs=xs[:, d, :],
                            start=(d == 0), stop=(d == D_T - 1),
                        )
                    g = gpool.tile([P, NTOK], bf16, tag="g", name="g")
                    nc.scalar.activation(g[:, :n], pg[:, :n], Silu)
                    ht = hpool.tile([P, NTOK], bf16, tag=f"h_{s}", name=f"h_{s}")
                    nc.vector.tensor_mul(out=ht[:, :n], in0=g[:, :n], in1=pu[:, :n])
                    hts.append(ht)

                # stage 2: yT[do] = sum_s w2s[s,do].T @ hT[s]  [128, n] bf16
                # All outputs ride the ACT ring -- it stays warm from
                # streaming outputs all along, while the SP ring is cold-idle
                # by the last chunk and would pay ~2us of DGE re-spin-up.
                for do in range(D_T):
                    py = pspool.tile([P, NTOK], f32, tag="py", name="py", bufs=3)
                    for s in range(HS_T):
                        nc.tensor.matmul(
                            py[:, :n],
                            lhsT=t2[:, s, do * P:(do + 1) * P],
                            rhs=hts[s][:, :n],
                            start=(s == 0), stop=(s == HS_T - 1),
                        )
                    # bufs=8: the up-front input flood keeps the 16 HW DMA
                    # queues busy for the first ~55us, starving the ACT
                    # ring's output DMAs; a full chunk of copies must be able
                    # to land without waiting on any output-DMA completion,
                    # else the py->copy->ot chain backpressures the PE.
                    ot = opool.tile([P, NTOK], bf16, tag="o", name="o", bufs=8)
                    nc.vector.tensor_copy(ot[:, :n], py[:, :n])
                    nc.scalar.dma_start(yT_d[do * P:(do + 1) * P, s0:s0 + n], ot[:, :n])

    nc.finalize()
    return nc


def kernel(x, router_w, w1, w2, w3):
    global LAST_RESULTS
    from concourse.bass_utils import run_bass_kernel_spmd

    x = np.ascontiguousarray(np.asarray(x, dtype=np.float32))
    router_w = np.asarray(router_w, dtype=np.float32)
    flat = x.reshape(T, D)

    # ---- host router (fp32, matches reference math) ----
    logits = flat @ router_w.T                      # [T, E]
    rows = np.arange(T)
    i1 = np.argmax(logits, axis=1)
    v1 = logits[rows, i1]
    masked = logits.copy()
    masked[rows, i1] = -np.inf
    i2 = np.argmax(masked, axis=1)
    v2 = masked[rows, i2]
    e2 = np.exp(v2 - v1)
    wt1 = 1.0 / (1.0 + e2)
    wt2 = e2 / (1.0 + e2)

    # ---- dispatch: token lists per expert ----
    idxs, wts = [], []
    for e in range(E):
        m1 = i1 == e
        m2 = i2 == e
        idx = np.nonzero(m1 | m2)[0]
        w = np.where(m1[idx], wt1[idx], wt2[idx]).astype(np.float32)
        idxs.append(idx)
        wts.append(w)
    cnts = np.array([len(i) for i in idxs])

    # pair big-with-small so both group capacities stay tight
    order = np.argsort(-cnts)
    pairs = [(int(order[i]), int(order[E - 1 - i])) for i in range(NPAIR)]
    C1 = -(-int(cnts[order[0]]) // 8) * 8
    C2 = -(-int(max(cnts[e2_] for _, e2_ in pairs)) // 8) * 8
    caps = (C1, C2)

    if caps not in _cache:
        _cache[caps] = _build_nc(caps)
    nc = _cache[caps]

    # ---- inputs (bf16, pre-transposed, pre-packed) ----
    bf = ml_dtypes.bfloat16
    chunks = _chunk_sizes(C1) + _chunk_sizes(C2)
    xpacks = []
    for eA, eB in pairs:
        xT = np.zeros((D, C1 + C2), dtype=bf)
        xT[:, :cnts[eA]] = flat[idxs[eA]].T.astype(bf)
        xT[:, C1:C1 + cnts[eB]] = flat[idxs[eB]].T.astype(bf)
        xpacks.append(_pack_x(xT, chunks))

    in_maps = []
    for c in range(E):
        p, hh = c // TP, c % TP
        r0 = hh * HS
        w13 = np.empty((2 * HS_T * 2, P, D_T * P), dtype=bf)
        w2s = np.empty((2, P, HS_T * D), dtype=bf)
        for slot, e in enumerate(pairs[p]):
            w1eT = np.ascontiguousarray(w1[e][r0:r0 + HS].T).astype(bf)  # [D, HS]
            w3eT = np.ascontiguousarray(w3[e][r0:r0 + HS].T).astype(bf)
            a1 = w1eT.reshape(D_T, P, HS).transpose(1, 0, 2)   # [128, D_T, HS]
            a3 = w3eT.reshape(D_T, P, HS).transpose(1, 0, 2)
            for s in range(HS_T):
                w13[slot * HS_T * 2 + s * 2 + 0] = \
                    a1[:, :, s * P:(s + 1) * P].reshape(P, D_T * P)
                w13[slot * HS_T * 2 + s * 2 + 1] = \
                    a3[:, :, s * P:(s + 1) * P].reshape(P, D_T * P)
            w2eT = np.ascontiguousarray(w2[e].T[r0:r0 + HS]).astype(bf)  # [HS, D]
            w2s[slot] = w2eT.reshape(HS_T, P, D).transpose(1, 0, 2).reshape(P, HS_T * D)
        in_maps.append({"xT": xpacks[p], "w13T": w13, "w2T": w2s})

    trace = os.environ.get("KERNEL_TRACE", "0") == "1"
    kwargs = {}
    if trace:
        kwargs = dict(trace=True, trace_cores=list(range(E)))
    res = run_bass_kernel_spmd(nc, in_maps, core_ids=list(range(E)), **kwargs)
    LAST_RESULTS = res

    # ---- combine: sum the two hidden-half partials, weight, scatter ----
    out = np.zeros((T, D), dtype=np.float32)
    for p, (eA, eB) in enumerate(pairs):
        ysum = (res.results[p * TP]["yT"].astype(np.float32)
                + res.results[p * TP + 1]["yT"].astype(np.float32))
        out[idxs[eA]] += wts[eA][:, None] * ysum[:, :cnts[eA]].T
        out[idxs[eB]] += wts[eB][:, None] * ysum[:, C1:C1 + cnts[eB]].T
    return out.reshape(B, S, D)
